# revision 9
# baseline (speedup 1.0000x reference)
"""LIF (leaky integrate-and-fire) forward kernel for Trainium2, 8 NeuronCores.

Recurrence (per element of [B, N], serial over T):
    v_t = DECAY * w_{t-1} + x_t          (REST = 0, w = post-reset membrane)
    s_t = (v_t > THRESHOLD)
    w_t = v_t * (v_t <= THRESHOLD)

Engine plan (per core, per step tile of [128 partitions, 2048]):
  - Columns are split into two independent recurrence lanes:
    DVE owns cols [0, FD), GpSimd owns cols [FD, 2048). Each lane runs both
    fused scalar_tensor_tensor ops of its own block, so the serial t-chain
    runs on two engines in parallel instead of one.
  - ScalarE emits the spike as Sign(v - THR) in fp8 {-1, 0, 1}.
  - PE packs 8 consecutive steps' signs into one byte-plane: accumulating
    matmuls with stationary weights 2^k * I (fp8) into PSUM, then ScalarE
    converts (psum + 255)/2 -> uint8. Output traffic drops 8x vs fp8 spikes.
  - Host decodes bit k of each byte as the spike at t = 8*g + k.

All recurrence arithmetic is fp32 and bitwise-faithful to the reference
ordering. (A byte can only be corrupted if some v_t == THR exactly, which
Sign maps to 0; measure-zero in practice and far inside the error budget.)

Sharding: batch dim (128) split 16 rows/core across 8 cores; per-core,
per-step slab is a contiguous 1 MiB block viewed as [128 partitions, 2048].
"""

import numpy as np

import concourse.bacc as bacc
import concourse.mybir as mybir
from concourse.tile import TileContext
from concourse.bass_utils import run_bass_kernel_spmd

T, B, N = 32, 128, 16384
N_CORES = 8
B_SH = B // N_CORES          # 16 batch rows per core
S = B_SH * N                 # 262144 elements per core per time step
P = 128                      # SBUF partitions
F = S // P                   # 2048 free-dim elements
FD = 1344                    # DVE-owned columns; GpSimd owns F - FD
G = T // 8                   # packed byte groups
DECAY = 0.2
THR = 0.3

TRACE = False                # set True (e.g. from test.py) to capture a profile

_BUILT = {}


def _build_nc():
    nc = bacc.Bacc("TRN2", debug=False, num_devices=N_CORES)
    x = nc.dram_tensor("x", [T, S], mybir.dt.float32, kind="ExternalInput").ap()
    y = nc.dram_tensor("y", [G, S], mybir.dt.uint8, kind="ExternalOutput").ap()
    xr = x.rearrange("t (p f) -> t p f", p=P)
    yr = y.rearrange("g (p f) -> g p f", p=P)

    f32 = mybir.dt.float32
    fp8 = mybir.dt.float8e4
    Alu = mybir.AluOpType
    Act = mybir.ActivationFunctionType

    with TileContext(nc) as tc:
        with (
            tc.tile_pool(name="state", bufs=1) as state_pool,
            tc.tile_pool(name="xin", bufs=8) as xin_pool,
            tc.tile_pool(name="va", bufs=2) as va_pool,
            tc.tile_pool(name="vb", bufs=2) as vb_pool,
            tc.tile_pool(name="mb", bufs=2) as mb_pool,
            tc.tile_pool(name="st", bufs=3) as st_pool,
            tc.tile_pool(name="ob", bufs=2) as ob_pool,
            tc.tile_pool(name="pk", bufs=2, space="PSUM") as psum_pool,
        ):
            negthr = nc.alloc_sbuf_tensor("const_negthr", [P, 1], f32).ap()
            nc.vector.memset(negthr, -THR)

            # Pack weights: wk[k] = 2^k * I in fp8 (diag via affine_select
            # on an iota p - f, equal-0 keeps in_, else fill 0). Persistent
            # allocations: all 8 must stay alive for the whole kernel.
            wtmp = nc.alloc_sbuf_tensor("wk_tmp", [P, 128], f32).ap()
            wks = []
            for k in range(8):
                wk = nc.alloc_sbuf_tensor(f"wk_{k}", [P, 128], fp8).ap()
                nc.vector.memset(wtmp, float(1 << k))
                nc.gpsimd.affine_select(
                    out=wk, in_=wtmp, pattern=[[-1, 128]],
                    compare_op=Alu.is_equal, fill=0.0,
                    base=0, channel_multiplier=1,
                )
                wks.append(wk)

            # lane A state: wA = post-reset membrane (DVE, 2 fused STTs/step)
            # lane B state: uB = DECAY * post-reset membrane (Pool lacks STT;
            #   3 ops/step: v = u + x; m = (v<=THR)*DECAY; u = m*v)
            wA = state_pool.tile([P, FD], f32)
            uB = state_pool.tile([P, F - FD], f32)

            ps = None
            for t in range(T):
                g, k = divmod(t, 8)
                xt = xin_pool.tile([P, F], f32)
                if t == 0:
                    # split the first load so compute can start sooner
                    nc.sync.dma_start(out=xt[:, :FD], in_=xr[t][:, :FD])
                    nc.sync.dma_start(out=xt[:, FD:], in_=xr[t][:, FD:])
                else:
                    nc.sync.dma_start(out=xt[:], in_=xr[t])

                st = st_pool.tile([P, F], fp8)
                if t == 0:
                    # w_{-1}=0 so v_0 = x_0: read x directly
                    nc.vector.scalar_tensor_tensor(
                        out=wA[:], in0=xt[:, :FD], scalar=THR,
                        in1=xt[:, :FD], op0=Alu.is_le, op1=Alu.mult,
                    )
                    mB0 = mb_pool.tile([P, F - FD], f32)
                    nc.gpsimd.tensor_scalar(
                        out=mB0[:], in0=xt[:, FD:], scalar1=THR, scalar2=DECAY,
                        op0=Alu.is_le, op1=Alu.mult,
                    )
                    nc.gpsimd.tensor_tensor(
                        out=uB[:], in0=mB0[:], in1=xt[:, FD:], op=Alu.mult,
                    )
                    nc.scalar.activation(st[:, :FD], xt[:, :FD], Act.Sign, bias=negthr)
                    nc.scalar.activation(st[:, FD:], xt[:, FD:], Act.Sign, bias=negthr)
                else:
                    vA = va_pool.tile([P, FD], f32)
                    vB = vb_pool.tile([P, F - FD], f32)
                    # lane A (DVE): v = w*DECAY + x ; w = (v<=THR)*v
                    nc.vector.scalar_tensor_tensor(
                        out=vA[:], in0=wA[:], scalar=DECAY, in1=xt[:, :FD],
                        op0=Alu.mult, op1=Alu.add,
                    )
                    nc.vector.scalar_tensor_tensor(
                        out=wA[:], in0=vA[:], scalar=THR, in1=vA[:],
                        op0=Alu.is_le, op1=Alu.mult,
                    )
                    # lane B (Pool): v = u + x ; m = (v<=THR)*DECAY ; u = m*v
                    nc.gpsimd.tensor_tensor(
                        out=vB[:], in0=uB[:], in1=xt[:, FD:], op=Alu.add,
                    )
                    mB = mb_pool.tile([P, F - FD], f32)
                    nc.gpsimd.tensor_scalar(
                        out=mB[:], in0=vB[:], scalar1=THR, scalar2=DECAY,
                        op0=Alu.is_le, op1=Alu.mult,
                    )
                    nc.gpsimd.tensor_tensor(
                        out=uB[:], in0=mB[:], in1=vB[:], op=Alu.mult,
                    )
                    nc.scalar.activation(st[:, :FD], vA[:], Act.Sign, bias=negthr)
                    nc.scalar.activation(st[:, FD:], vB[:], Act.Sign, bias=negthr)

                # pack: psum[:, bank j] += 2^k * st  (identity-scaled matmul)
                if k == 0:
                    ps = psum_pool.tile([P, F], f32)
                for j in range(0, F, 512):
                    nc.tensor.matmul(
                        out=ps[:, j:j + 512], lhsT=wks[k][:], rhs=st[:, j:j + 512],
                        start=(k == 0), stop=(k == 7),
                    )
                if k == 7:
                    ob = ob_pool.tile([P, F], mybir.dt.uint8)
                    # (sum_k 2^k sign_k + 255) / 2 -> byte of spike bits
                    nc.scalar.activation(ob[:], ps[:], Act.Copy, bias=127.5, scale=0.5)
                    nc.scalar.dma_start(out=yr[g], in_=ob[:])
    nc.compile()
    return nc


LAST_RESULTS = None


def kernel(tx):
    global LAST_RESULTS
    tx = np.asarray(tx)
    assert tx.shape == (T, B, N) and tx.dtype == np.float32

    if "nc" not in _BUILT:
        _BUILT["nc"] = _build_nc()
    nc = _BUILT["nc"]

    in_maps = [
        {"x": np.ascontiguousarray(tx[:, c * B_SH:(c + 1) * B_SH, :]).reshape(T, S)}
        for c in range(N_CORES)
    ]
    res = run_bass_kernel_spmd(nc, in_maps, core_ids=list(range(N_CORES)), trace=TRACE)
    LAST_RESULTS = res

    out = np.empty((T, B, N), dtype=np.float32)
    for c in range(N_CORES):
        packed = np.asarray(res.results[c]["y"]).reshape(G, B_SH, N, 1)
        bits = np.unpackbits(packed, axis=3, bitorder="little")  # [G, B_SH, N, 8]
        sp = np.moveaxis(bits, 3, 1).reshape(T, B_SH, N)
        out[:, c * B_SH:(c + 1) * B_SH, :] = sp
    return out


# revision 10
# speedup vs baseline: 2.9709x; 2.9709x over previous
"""LIF (leaky integrate-and-fire) forward kernel for Trainium2, 8 NeuronCores.

Recurrence (per element of [B, N], serial over T):
    v_t = DECAY * w_{t-1} + x_t          (REST = 0, w = post-reset membrane)
    s_t = (v_t > THRESHOLD)
    w_t = v_t * (v_t <= THRESHOLD)

Engine plan (per core, per step tile of [128 partitions, 2048]):
  - DVE: the two fused scalar_tensor_tensor ops of the recurrence (the
    serial chain; ~2.2us each, the kernel's critical path).
  - ScalarE: spike as Sign(v - THR) in fp8 {-1, 0, 1}.
  - PE: packs 8 consecutive steps' signs into one byte-plane via
    accumulating matmuls with stationary weights 2^k * I (fp8) into PSUM;
    ScalarE then converts (psum + 255)/2 -> uint8. Output traffic is 8x
    smaller than storing fp8 spikes, keeping DMA well under the chain.
  - Host decodes bit k of byte-plane g as the spike at t = 8*g + k.

All recurrence arithmetic is fp32 and bitwise-faithful to the reference
ordering. (A byte can only be corrupted if some v_t == THR exactly, which
Sign maps to 0; measure-zero in practice and far inside the 2e-2 budget.)

Sharding: batch dim (128) split 16 rows/core across 8 cores; per-core,
per-step slab is a contiguous 1 MiB block viewed as [128 partitions, 2048].
"""

import numpy as np

import concourse.bacc as bacc
import concourse.mybir as mybir
from concourse.tile import TileContext
from concourse.bass_utils import run_bass_kernel_spmd

T, B, N = 32, 128, 16384
N_CORES = 8
B_SH = B // N_CORES          # 16 batch rows per core
S = B_SH * N                 # 262144 elements per core per time step
P = 128                      # SBUF partitions
F = S // P                   # 2048 free-dim elements
G = T // 8                   # packed byte groups
DECAY = 0.2
THR = 0.3

TRACE = False                # set True (e.g. from test.py) to capture a profile

_BUILT = {}


def _build_nc():
    nc = bacc.Bacc("TRN2", debug=False, num_devices=N_CORES)
    x = nc.dram_tensor("x", [T, S], mybir.dt.float32, kind="ExternalInput").ap()
    y = nc.dram_tensor("y", [G, S], mybir.dt.uint8, kind="ExternalOutput").ap()
    xr = x.rearrange("t (p f) -> t p f", p=P)
    yr = y.rearrange("g (p f) -> g p f", p=P)

    f32 = mybir.dt.float32
    fp8 = mybir.dt.float8e4
    Alu = mybir.AluOpType
    Act = mybir.ActivationFunctionType

    H = F // 2
    with TileContext(nc) as tc:
        with (
            tc.tile_pool(name="state", bufs=1) as state_pool,
            tc.tile_pool(name="xin", bufs=10) as xin_pool,
            tc.tile_pool(name="vtmp", bufs=3) as v_pool,
            tc.tile_pool(name="st", bufs=3) as st_pool,
            tc.tile_pool(name="ob", bufs=2) as ob_pool,
            tc.tile_pool(name="pk", bufs=2, space="PSUM") as psum_pool,
        ):
            negthr = nc.alloc_sbuf_tensor("const_negthr", [P, 1], f32).ap()
            nc.vector.memset(negthr, -THR)

            # Pack weights: wk[k] = 2^k * I in fp8 (diag via affine_select on
            # the iota p - f; equal-0 keeps in_, else fill 0). Persistent
            # allocations: all 8 must stay alive for the whole kernel.
            wtmp = nc.alloc_sbuf_tensor("wk_tmp", [P, 128], f32).ap()
            wks = []
            for k in range(8):
                wk = nc.alloc_sbuf_tensor(f"wk_{k}", [P, 128], fp8).ap()
                nc.vector.memset(wtmp, float(1 << k))
                nc.gpsimd.affine_select(
                    out=wk, in_=wtmp, pattern=[[-1, 128]],
                    compare_op=Alu.is_equal, fill=0.0,
                    base=0, channel_multiplier=1,
                )
                wks.append(wk)

            w = state_pool.tile([P, F], f32)

            ps = None
            for t in range(T):
                g, k = divmod(t, 8)
                xt = xin_pool.tile([P, F], f32)
                if t == 0:
                    # split the first load so compute can start sooner
                    nc.sync.dma_start(out=xt[:, :H], in_=xr[t][:, :H])
                    nc.sync.dma_start(out=xt[:, H:], in_=xr[t][:, H:])
                else:
                    nc.sync.dma_start(out=xt[:], in_=xr[t])

                st = st_pool.tile([P, F], fp8)
                if t == 0:
                    # w_{-1}=0 so v_0 = x_0: read x directly
                    for c0, c1 in ((0, H), (H, F)):
                        nc.vector.scalar_tensor_tensor(
                            out=w[:, c0:c1], in0=xt[:, c0:c1], scalar=THR,
                            in1=xt[:, c0:c1], op0=Alu.is_le, op1=Alu.mult,
                        )
                        nc.scalar.activation(
                            st[:, c0:c1], xt[:, c0:c1], Act.Sign, bias=negthr
                        )
                else:
                    v = v_pool.tile([P, F], f32)
                    # v = w*DECAY + x
                    nc.vector.scalar_tensor_tensor(
                        out=v[:], in0=w[:], scalar=DECAY, in1=xt[:],
                        op0=Alu.mult, op1=Alu.add,
                    )
                    # w = (v<=THR)*v -- dead at the last step, skip there
                    if t < T - 1:
                        nc.vector.scalar_tensor_tensor(
                            out=w[:], in0=v[:], scalar=THR, in1=v[:],
                            op0=Alu.is_le, op1=Alu.mult,
                        )
                    nc.scalar.activation(st[:], v[:], Act.Sign, bias=negthr)

                # pack: psum bank j accumulates 2^k * st (identity matmul)
                if k == 0:
                    ps = psum_pool.tile([P, F], f32)
                for j in range(0, F, 512):
                    nc.tensor.matmul(
                        out=ps[:, j:j + 512], lhsT=wks[k][:], rhs=st[:, j:j + 512],
                        start=(k == 0), stop=(k == 7),
                    )
                if k == 7:
                    ob = ob_pool.tile([P, F], mybir.dt.uint8)
                    # (sum_k 2^k sign_k + 255) / 2 -> byte of spike bits
                    nc.scalar.activation(ob[:], ps[:], Act.Copy, bias=127.5, scale=0.5)
                    nc.scalar.dma_start(out=yr[g], in_=ob[:])
    nc.compile()
    return nc


LAST_RESULTS = None


def kernel(tx):
    global LAST_RESULTS
    tx = np.asarray(tx)
    assert tx.shape == (T, B, N) and tx.dtype == np.float32

    if "nc" not in _BUILT:
        _BUILT["nc"] = _build_nc()
    nc = _BUILT["nc"]

    in_maps = [
        {"x": np.ascontiguousarray(tx[:, c * B_SH:(c + 1) * B_SH, :]).reshape(T, S)}
        for c in range(N_CORES)
    ]
    res = run_bass_kernel_spmd(nc, in_maps, core_ids=list(range(N_CORES)), trace=TRACE)
    LAST_RESULTS = res

    out = np.empty((T, B, N), dtype=np.float32)
    for c in range(N_CORES):
        packed = np.asarray(res.results[c]["y"]).reshape(G, B_SH, N, 1)
        bits = np.unpackbits(packed, axis=3, bitorder="little")  # [G, B_SH, N, 8]
        sp = np.moveaxis(bits, 3, 1).reshape(T, B_SH, N)
        out[:, c * B_SH:(c + 1) * B_SH, :] = sp
    return out
